# revision 14
# baseline (speedup 1.0000x reference)
"""Trainium2 Bass kernel for CausalSelfAttention (GQA + qk-rmsnorm + rope + head gating).

Sharding: 8 cores = 2 (batch) x 4 (kv-head groups). Each core computes the
full attention for one batch element and one kv-head group (4 q heads), plus
its slice of the output projection; partial projection outputs are summed on
the host (bf16 partials, fp32 sum).

Per-core on-device pipeline (all matmuls bf16 with fp32 PSUM accumulation):
  A) fused QKV+gate projection -> rmsnorm stats + rope (DVE/ACT) ->
     DMA-transpose q,k into head-dim-major layout.
     Input DMAs are contiguous and interleaved across the sync+scalar
     hardware queues; phase A runs chunk-outer over pairs of token tiles
     (the first two pairs interleaved chunk-major) so the PE tracks input
     DMA arrival instead of waiting for all of it.
  B) flash-style causal attention per head in S^T layout:
     S^T = K @ Q^T, P = exp(S/sqrt(d)) (no max subtraction: |logits| <= 11.3),
     diagonal-block masking, Y = P @ [V | 1] (ones column gives the softmax
     denominator for free). S matmuls are emitted one tile ahead of the P@V
     matmuls so the in-order tensor queue never waits on the scalar exp.
     Per-token normalize fuses the sigmoid gate as 1/(denom*(1+exp(-glog)))
     so the only scalar tables are Sqrt (phase A) and Exp (phase B).
  C) output projection partial: out = y @ Wproj_slice^T, stored bf16.
     C(qc-1) tile blocks are interleaved between B(qc) head blocks to give
     the tensor queue exp-independent work.
"""

import numpy as np
import ml_dtypes
from contextlib import ExitStack

import concourse.bass as bass
import concourse.bacc as bacc
import concourse.mybir as mybir
import concourse.tile as tile
from concourse.bass_utils import run_bass_kernel_spmd

BF16 = mybir.dt.bfloat16
F32 = mybir.dt.float32
NPBF = ml_dtypes.bfloat16

B, T, D = 2, 2048, 2048
H, HKV, HD = 16, 4, 128
HALF = HD // 2
NHEAD = H // HKV          # q heads per core (group)
NT = T // 128             # 16 token tiles
NCHUNK = D // 128         # 16 contraction chunks
NQKV = NHEAD * HD + HD + HD + NHEAD   # 512 q + 128 k + 128 v + 4 gate = 772
ROPE_BASE = 10000.0
EPS = float(np.finfo(np.float32).eps)
SM_SCALE = 1.0 / float(np.sqrt(HD))

_CACHE = {}


def _build_program():
    nc = bacc.Bacc("TRN2", target_bir_lowering=False, debug=False,
                   enable_asserts=False, num_devices=8)

    xT_d = nc.dram_tensor("xT", [D, T], BF16, kind="ExternalInput").ap()
    wqkvg_d = nc.dram_tensor("wqkvg", [D, NQKV], BF16, kind="ExternalInput").ap()
    wproj_d = nc.dram_tensor("wproj", [NHEAD * HD, D], BF16, kind="ExternalInput").ap()
    cos_d = nc.dram_tensor("cosd", [128, NT * HALF], F32, kind="ExternalInput").ap()
    sin_d = nc.dram_tensor("sind", [128, NT * HALF], F32, kind="ExternalInput").ap()
    qgain_d = nc.dram_tensor("qgain", [128, NHEAD], F32, kind="ExternalInput").ap()
    gateb_d = nc.dram_tensor("gateb", [128, NHEAD], F32, kind="ExternalInput").ap()
    mask_d = nc.dram_tensor("mask", [128, 128], BF16, kind="ExternalInput").ap()
    out_d = nc.dram_tensor("out", [T, D], BF16, kind="ExternalOutput").ap()

    AF = mybir.ActivationFunctionType

    with tile.TileContext(nc) as tc, ExitStack() as ctx:
        consts = ctx.enter_context(tc.tile_pool(name="consts", bufs=1))

        # ---- input DMAs: first x/w chunk pair leads on both hardware queues
        # so the PE can start ~10us in; small constants ride behind chunk 2.
        xT_sb = consts.tile([128, NCHUNK, T], BF16)
        wqkvg_sb = consts.tile([128, NCHUNK, NQKV], BF16)
        cos_sb = consts.tile([128, NT, HALF], F32)
        sin_sb = consts.tile([128, NT, HALF], F32)
        qgain_sb = consts.tile([128, NHEAD], F32)
        gateb_sb = consts.tile([128, NHEAD], F32)
        mask_sb = consts.tile([128, 128], BF16)

        def load_chunk(c):
            qx = nc.sync if c % 2 == 0 else nc.scalar
            qw = nc.scalar if c % 2 == 0 else nc.sync
            qx.dma_start(out=xT_sb[:, c, :], in_=xT_d[c * 128:(c + 1) * 128, :])
            qw.dma_start(out=wqkvg_sb[:, c, :],
                         in_=wqkvg_d[c * 128:(c + 1) * 128, :])

        for c in range(3):
            load_chunk(c)
        nc.scalar.dma_start(out=cos_sb.rearrange("p a b -> p (a b)"), in_=cos_d)
        nc.sync.dma_start(out=sin_sb.rearrange("p a b -> p (a b)"), in_=sin_d)
        nc.sync.dma_start(out=qgain_sb, in_=qgain_d)
        nc.sync.dma_start(out=gateb_sb, in_=gateb_d)
        nc.scalar.dma_start(out=mask_sb, in_=mask_d)
        for c in range(3, NCHUNK):
            load_chunk(c)
        wproj_sb = consts.tile([128, NHEAD, D], BF16)

        qT_sb = consts.tile([128, NHEAD, T], BF16)   # head-dim-major q
        kT_sb = consts.tile([128, T], BF16)          # head-dim-major k
        v_sb = consts.tile([128, NT, HD + 1], BF16)  # [t | ones] per ki tile
        nc.vector.memset(v_sb[:, :, HD:HD + 1], 1.0)
        yT_sb = consts.tile([128, NHEAD, T], BF16)   # head-dim-major gated y
        glog_all = consts.tile([128, NT, NHEAD], F32)
        egp1_all = consts.tile([128, NT, NHEAD], F32)  # 1 + exp(-glog)
        eps_sb = consts.tile([128, 1], F32)
        nc.vector.memset(eps_sb, EPS)

        # =========== Phase A: QKV + gate, rms stats, rope, transpose ==========
        a_sb = ctx.enter_context(tc.tile_pool(name="phA", bufs=2))

        def emit_tile_post(tt, ti, qkv_a, qkv_b, msq_g, qst_g, kst_g):
            """Vector-side post-processing for one 128-token tile: v copy,
            gate logits, rope on q and k, per-head mean-square stats."""
            nc.vector.tensor_copy(out=v_sb[:, tt, 0:HD], in_=qkv_b[:, 128:256])
            nc.vector.tensor_add(glog_all[:, tt, :], qkv_b[:, 256:260], gateb_sb)

            # rope on q (all 4 heads at once via broadcast cos/sin)
            qa3 = qkv_a.rearrange("p (h d) -> p h d", h=NHEAD)
            x1 = qa3[:, :, 0:HALF]
            x2 = qa3[:, :, HALF:HD]
            cos_t = cos_sb[:, tt, :]
            sin_t = sin_sb[:, tt, :]
            cos_b = bass.AP(tensor=cos_t.tensor, offset=cos_t.offset,
                            ap=[cos_t.ap[0], [0, NHEAD], cos_t.ap[1]])
            sin_b = bass.AP(tensor=sin_t.tensor, offset=sin_t.offset,
                            ap=[sin_t.ap[0], [0, NHEAD], sin_t.ap[1]])
            qrot = qst_g[:, ti, :, :]
            u1 = a_sb.tile([128, NHEAD, HALF], F32, tag="u1")
            u2 = a_sb.tile([128, NHEAD, HALF], F32, tag="u2")
            nc.vector.tensor_mul(u1, x1, cos_b)
            nc.vector.tensor_mul(u2, x2, sin_b)
            nc.vector.tensor_add(qrot[:, :, 0:HALF], u1, u2)
            u3 = a_sb.tile([128, NHEAD, HALF], F32, tag="u3")
            u4 = a_sb.tile([128, NHEAD, HALF], F32, tag="u4")
            nc.vector.tensor_mul(u3, x2, cos_b)
            nc.vector.tensor_mul(u4, x1, sin_b)
            nc.vector.tensor_sub(qrot[:, :, HALF:HD], u3, u4)
            # rope on k
            k1 = qkv_b[:, 0:HALF]
            k2 = qkv_b[:, HALF:HD]
            krot = kst_g[:, ti, :]
            w1 = a_sb.tile([128, HALF], F32, tag="w1")
            w2 = a_sb.tile([128, HALF], F32, tag="w2")
            nc.vector.tensor_mul(w1, k1, cos_t)
            nc.vector.tensor_mul(w2, k2, sin_t)
            nc.vector.tensor_add(krot[:, 0:HALF], w1, w2)
            nc.vector.tensor_mul(w1, k2, cos_t)
            nc.vector.tensor_mul(w2, k1, sin_t)
            nc.vector.tensor_sub(krot[:, HALF:HD], w1, w2)

            # mean-square per head from the (norm-preserving) rotated values
            sqscr = a_sb.tile([128, NHEAD, HD], F32, tag="sqscr")
            sqscr_k = a_sb.tile([128, HD], F32, tag="sqscr_k")
            nc.vector.tensor_mul(sqscr, qrot, qrot)
            nc.vector.tensor_reduce(msq_g[:, ti, 0:NHEAD], sqscr,
                                    axis=mybir.AxisListType.X,
                                    op=mybir.AluOpType.add)
            nc.vector.tensor_mul(sqscr_k, krot, krot)
            nc.vector.tensor_reduce(msq_g[:, ti, NHEAD:NHEAD + 1], sqscr_k,
                                    axis=mybir.AxisListType.X,
                                    op=mybir.AluOpType.add)

        def emit_pair_finish(pr, msq_g, qst_g, kst_g):
            """Rms scale + gain for one pair of tiles + DMA transposes.
            Scalar only runs Sqrt in phase A (table stays resident)."""
            rtmp_g = a_sb.tile([128, 2, NHEAD + 1], F32, tag="rtmp_g")
            nc.scalar.activation(out=rtmp_g, in_=msq_g, func=AF.Sqrt,
                                 scale=1.0 / HD, bias=eps_sb)
            rinv_g = a_sb.tile([128, 2, NHEAD + 1], F32, tag="rinv_g")
            nc.vector.reciprocal(rinv_g, rtmp_g)
            rq_g = a_sb.tile([128, 2, NHEAD], F32, tag="rq_g")
            for ti in range(2):
                nc.vector.tensor_mul(rq_g[:, ti, :], rinv_g[:, ti, 0:NHEAD],
                                     qgain_sb)
            for ti in range(2):
                tt = pr * 2 + ti
                ts = slice(tt * 128, (tt + 1) * 128)
                k_stage = a_sb.tile([128, HD], BF16, tag="k_stage")
                nc.vector.tensor_scalar_mul(k_stage, kst_g[:, ti, :],
                                            rinv_g[:, ti, NHEAD:NHEAD + 1])
                q_stage = a_sb.tile([128, NHEAD, HD], BF16, tag="q_stage")
                for h in range(NHEAD):
                    nc.vector.tensor_scalar_mul(q_stage[:, h, :],
                                                qst_g[:, ti, h, :],
                                                rq_g[:, ti, h:h + 1])
                nc.sync.dma_start_transpose(out=qT_sb[:, :, ts], in_=q_stage)
                nc.sync.dma_start_transpose(out=kT_sb[:, ts], in_=k_stage)

        NPAIR = NT // 2
        with tc.tile_pool(name="phA_ps", bufs=2, space="PSUM") as a_ps:
            # wproj: needed only by phase C; queue behind phase-A loads
            for h in range(NHEAD):
                nc.sync.dma_start(out=wproj_sb[:, h, :],
                                  in_=wproj_d[h * 128:(h + 1) * 128, :])

            # pairs 0..7: chunk-outer, double-buffered; pair 0 tracks the
            # arrival of the input chunk DMAs
            for pr in range(NPAIR):
                qa = a_ps.tile([128, 2, 512], F32, tag="qkv_a")
                qb = a_ps.tile([128, 2, 512], F32, tag="qkv_b")
                for c in range(NCHUNK):
                    for ti in range(2):
                        tt = pr * 2 + ti
                        lhs = xT_sb[:, c, tt * 128:(tt + 1) * 128]
                        nc.tensor.matmul(qa[:, ti, :], lhsT=lhs,
                                         rhs=wqkvg_sb[:, c, 0:512],
                                         start=(c == 0), stop=(c == NCHUNK - 1))
                        nc.tensor.matmul(qb[:, ti, 0:NQKV - 512], lhsT=lhs,
                                         rhs=wqkvg_sb[:, c, 512:NQKV],
                                         start=(c == 0), stop=(c == NCHUNK - 1))
                msq_g = a_sb.tile([128, 2, NHEAD + 1], F32, tag="msq_g")
                qst_g = a_sb.tile([128, 2, NHEAD, HD], BF16, tag="qst_g")
                kst_g = a_sb.tile([128, 2, HD], BF16, tag="kst_g")
                for ti in range(2):
                    emit_tile_post(pr * 2 + ti, ti, qa[:, ti, :],
                                   qb[:, ti, 0:NQKV - 512],
                                   msq_g, qst_g, kst_g)
                emit_pair_finish(pr, msq_g, qst_g, kst_g)

        # =========== Phase B + C: attention, projection =======================
        b_sb = ctx.enter_context(tc.tile_pool(name="phB", bufs=3))
        c_sb = ctx.enter_context(tc.tile_pool(name="phC", bufs=3))

        def emit_C_block(qc, qs, b_ps):
            tt = qc * 4 + qs
            ts = slice(tt * 128, (tt + 1) * 128)
            for nch in range(4):
                o_ps = b_ps.tile([128, 512], F32, tag="o")
                for h in range(NHEAD):
                    nc.tensor.matmul(o_ps, lhsT=yT_sb[:, h, ts],
                                     rhs=wproj_sb[:, h, nch * 512:(nch + 1) * 512],
                                     start=(h == 0), stop=(h == NHEAD - 1))
                o_st = c_sb.tile([128, 512], BF16, tag="o_st")
                if nch % 2 == 0:
                    nc.scalar.copy(out=o_st, in_=o_ps)
                else:
                    nc.vector.tensor_copy(out=o_st, in_=o_ps)
                nc.sync.dma_start(out=out_d[ts, nch * 512:(nch + 1) * 512],
                                  in_=o_st)

        with tc.tile_pool(name="phBC_ps", bufs=2, space="PSUM") as b_ps:
            for qc in range(4):
                nki = 4 * qc + 4

                # gate for this qc's tiles: 1 + exp(-glog) (Exp table resident)
                eg = egp1_all[:, qc * 4:(qc + 1) * 4, :]
                eg_f = eg.rearrange("p a b -> p (a b)")
                nc.scalar.activation(
                    out=eg_f,
                    in_=glog_all[:, qc * 4:(qc + 1) * 4, :].rearrange(
                        "p a b -> p (a b)"),
                    func=AF.Exp, scale=-1.0)
                nc.vector.tensor_scalar_add(eg_f, eg_f, 1.0)

                def emit_S(h, ki):
                    """S^T matmul + exp (+ diagonal mask); returns p tile."""
                    m = ki - 4 * qc
                    nq = 512 - 128 * max(m, 0)
                    q_lo = qc * 512 + 128 * max(m, 0)
                    s_ps = b_ps.tile([128, 512], F32, tag="s")
                    nc.tensor.matmul(s_ps[:, 0:nq],
                                     lhsT=kT_sb[:, ki * 128:(ki + 1) * 128],
                                     rhs=qT_sb[:, h, q_lo:(qc + 1) * 512],
                                     start=True, stop=True)
                    p_sb = b_sb.tile([128, 512], BF16, tag="p")
                    nc.scalar.activation(out=p_sb[:, 0:nq], in_=s_ps[:, 0:nq],
                                         func=AF.Exp, scale=SM_SCALE)
                    if m >= 0:
                        # gpsimd: keeps masking off the busy DVE/ACT queues
                        nc.gpsimd.tensor_mul(p_sb[:, 0:128], p_sb[:, 0:128],
                                             mask_sb)
                    return p_sb

                for h in range(NHEAD):
                    y01 = b_ps.tile([128, 2, HD + 1], F32, tag="y01")
                    y23 = b_ps.tile([128, 2, HD + 1], F32, tag="y23")
                    # S emitted one tile ahead of P@V so the tensor queue
                    # always has independent work while exp(ki) runs.
                    p_cur = emit_S(h, 0)
                    for ki in range(nki):
                        p_next = emit_S(h, ki + 1) if ki + 1 < nki else None
                        m = ki - 4 * qc
                        for qs in range(max(m, 0), 4):
                            ytile = y01 if qs < 2 else y23
                            pcol = (qs - max(m, 0)) * 128
                            nc.tensor.matmul(
                                ytile[:, qs % 2, :],
                                lhsT=p_cur[:, pcol:pcol + 128],
                                rhs=v_sb[:, ki, :],
                                start=(ki == 0 and qs % 2 == 0),
                                stop=(ki == 4 * qc + qs and qs % 2 == 1))
                        p_cur = p_next
                    # normalize + fused sigmoid gate + transpose (scalar queue)
                    y_stage = b_sb.tile([128, 4, HD], BF16, tag="y_stage")
                    for qs in range(4):
                        ytile = y01 if qs < 2 else y23
                        tt = qc * 4 + qs
                        den = b_sb.tile([128, 1], F32, tag="den")
                        nc.vector.tensor_mul(den, egp1_all[:, tt, h:h + 1],
                                             ytile[:, qs % 2, HD:HD + 1])
                        sc = b_sb.tile([128, 1], F32, tag="sc")
                        nc.vector.reciprocal(sc, den)
                        nc.vector.tensor_scalar_mul(y_stage[:, qs, :],
                                                    ytile[:, qs % 2, 0:HD], sc)
                    yreg = yT_sb[:, h, qc * 512:(qc + 1) * 512]
                    y3d = bass.AP(tensor=yreg.tensor, offset=yreg.offset,
                                  ap=[yreg.ap[0], [128, 4], [1, 128]])
                    # alternate queues so the last transpose of a qc isn't
                    # stuck behind phase-C output stores
                    yq = nc.sync if h % 2 == 0 else nc.scalar
                    yq.dma_start_transpose(out=y3d, in_=y_stage)

                    # C for the previous qc rides between B head blocks
                    if qc >= 1:
                        emit_C_block(qc - 1, h, b_ps)

            for qs in range(4):
                emit_C_block(3, qs, b_ps)

    nc.compile()
    return nc


def _get_program():
    if "nc" not in _CACHE:
        _CACHE["nc"] = _build_program()
    return _CACHE["nc"]


def _host_prep(x, Wq, Wk, Wv, Wproj, q_gain, gate_w, gate_b):
    """Build the 8 per-core input maps."""
    f = np.float32
    x = np.asarray(x, f)
    WqT = np.asarray(Wq, f).T.astype(NPBF)       # [D, 2048]
    WkT = np.asarray(Wk, f).T.astype(NPBF)       # [D, 512]
    WvT = np.asarray(Wv, f).T.astype(NPBF)
    WpT = np.ascontiguousarray(np.asarray(Wproj, f).T.astype(NPBF))  # [D, D]
    gwT = np.asarray(gate_w, f).T.astype(NPBF)   # [D, 16]
    q_gain = np.asarray(q_gain, f)
    gate_b = np.asarray(gate_b, f)

    inv_freq = 1.0 / (ROPE_BASE ** (np.arange(0, HD, 2, dtype=f) / HD))
    tpos = np.arange(T, dtype=f)
    freqs = np.outer(tpos, inv_freq)
    # device layout [128 partitions, NT tiles, HALF] contiguous
    cos = np.ascontiguousarray(
        np.cos(freqs).astype(f).reshape(NT, 128, HALF).transpose(1, 0, 2)
    ).reshape(128, NT * HALF)
    sin = np.ascontiguousarray(
        np.sin(freqs).astype(f).reshape(NT, 128, HALF).transpose(1, 0, 2)
    ).reshape(128, NT * HALF)

    kloc = np.arange(128)[:, None]
    qloc = np.arange(128)[None, :]
    mask = (qloc >= kloc).astype(NPBF)           # [128, 128]

    xT = [np.ascontiguousarray(x[b].T).astype(NPBF) for b in range(B)]

    in_maps = []
    for core in range(8):
        b, g = divmod(core, 4)
        wqkvg = np.concatenate([
            WqT[:, 512 * g:512 * (g + 1)],
            WkT[:, 128 * g:128 * (g + 1)],
            WvT[:, 128 * g:128 * (g + 1)],
            gwT[:, NHEAD * g:NHEAD * (g + 1)],
        ], axis=1)                               # [D, 772]
        in_maps.append({
            "xT": xT[b],
            "wqkvg": np.ascontiguousarray(wqkvg),
            "wproj": np.ascontiguousarray(WpT[512 * g:512 * (g + 1), :]),
            "cosd": cos,
            "sind": sin,
            "qgain": np.ascontiguousarray(np.broadcast_to(
                q_gain[NHEAD * g:NHEAD * (g + 1)][None, :], (128, NHEAD))),
            "gateb": np.ascontiguousarray(np.broadcast_to(
                gate_b[NHEAD * g:NHEAD * (g + 1)][None, :], (128, NHEAD))),
            "mask": mask,
        })
    return in_maps


def kernel(**inputs):
    nc = _get_program()
    in_maps = _host_prep(**inputs)
    res = run_bass_kernel_spmd(nc, in_maps, list(range(8)))
    parts = [r["out"] for r in res.results]
    out = np.empty((B, T, D), np.float32)
    for b in range(B):
        out[b] = (parts[4 * b].astype(np.float32)
                  + parts[4 * b + 1].astype(np.float32)
                  + parts[4 * b + 2].astype(np.float32)
                  + parts[4 * b + 3].astype(np.float32))
    return out


# revision 15
# speedup vs baseline: 1.0460x; 1.0460x over previous
"""Trainium2 Bass kernel for CausalSelfAttention (GQA + qk-rmsnorm + rope + head gating).

Sharding: 8 cores = 2 (batch) x 4 (kv-head groups). Each core computes the
full attention for one batch element and one kv-head group (4 q heads), plus
its slice of the output projection; partial projection outputs are summed on
the host (bf16 partials, fp32 sum).

Per-core on-device pipeline (all matmuls bf16 with fp32 PSUM accumulation):
  A) fused QKV+gate projection -> rmsnorm stats + rope (DVE/ACT) ->
     DMA-transpose q,k into head-dim-major layout.
     Input DMAs are contiguous and interleaved across the sync+scalar
     hardware queues; phase A runs chunk-outer over pairs of token tiles
     (the first two pairs interleaved chunk-major) so the PE tracks input
     DMA arrival instead of waiting for all of it.
  B) flash-style causal attention per head in S^T layout:
     S^T = K @ Q^T, P = exp(S/sqrt(d)) (no max subtraction: |logits| <= 11.3),
     diagonal-block masking, Y = P @ [V | 1] (ones column gives the softmax
     denominator for free). S matmuls are emitted one tile ahead of the P@V
     matmuls so the in-order tensor queue never waits on the scalar exp.
     Per-token normalize fuses the sigmoid gate as 1/(denom*(1+exp(-glog)))
     so the only scalar tables are Sqrt (phase A) and Exp (phase B).
  C) output projection partial: out = y @ Wproj_slice^T, stored bf16.
     C(qc-1) tile blocks are interleaved between B(qc) head blocks to give
     the tensor queue exp-independent work.
"""

import numpy as np
import ml_dtypes
from contextlib import ExitStack

import concourse.bass as bass
import concourse.bacc as bacc
import concourse.mybir as mybir
import concourse.tile as tile
from concourse.bass_utils import run_bass_kernel_spmd

BF16 = mybir.dt.bfloat16
F32 = mybir.dt.float32
NPBF = ml_dtypes.bfloat16

B, T, D = 2, 2048, 2048
H, HKV, HD = 16, 4, 128
HALF = HD // 2
NHEAD = H // HKV          # q heads per core (group)
NT = T // 128             # 16 token tiles
NCHUNK = D // 128         # 16 contraction chunks
NQKV = NHEAD * HD + HD + HD + NHEAD   # 512 q + 128 k + 128 v + 4 gate = 772
ROPE_BASE = 10000.0
EPS = float(np.finfo(np.float32).eps)
SM_SCALE = 1.0 / float(np.sqrt(HD))

_CACHE = {}


def _build_program():
    nc = bacc.Bacc("TRN2", target_bir_lowering=False, debug=False,
                   enable_asserts=False, num_devices=8)

    xT_d = nc.dram_tensor("xT", [D, T], BF16, kind="ExternalInput").ap()
    wqkvg_d = nc.dram_tensor("wqkvg", [D, NQKV], BF16, kind="ExternalInput").ap()
    wproj_d = nc.dram_tensor("wproj", [NHEAD * HD, D], BF16, kind="ExternalInput").ap()
    cos_d = nc.dram_tensor("cosd", [128, NT * HALF], F32, kind="ExternalInput").ap()
    sin_d = nc.dram_tensor("sind", [128, NT * HALF], F32, kind="ExternalInput").ap()
    qgain_d = nc.dram_tensor("qgain", [128, NHEAD], F32, kind="ExternalInput").ap()
    gateb_d = nc.dram_tensor("gateb", [128, NHEAD], F32, kind="ExternalInput").ap()
    mask_d = nc.dram_tensor("mask", [128, 128], BF16, kind="ExternalInput").ap()
    out_d = nc.dram_tensor("out", [T, D], BF16, kind="ExternalOutput").ap()

    AF = mybir.ActivationFunctionType

    with tile.TileContext(nc) as tc, ExitStack() as ctx:
        consts = ctx.enter_context(tc.tile_pool(name="consts", bufs=1))

        # ---- input DMAs: first x/w chunk pair leads on both hardware queues
        # so the PE can start ~10us in; small constants ride behind chunk 2.
        xT_sb = consts.tile([128, NCHUNK, T], BF16)
        wqkvg_sb = consts.tile([128, NCHUNK, NQKV], BF16)
        cos_sb = consts.tile([128, NT, HALF], F32)
        sin_sb = consts.tile([128, NT, HALF], F32)
        qgain_sb = consts.tile([128, NHEAD], F32)
        gateb_sb = consts.tile([128, NHEAD], F32)
        mask_sb = consts.tile([128, 128], BF16)

        def load_chunk(c):
            qx = nc.sync if c % 2 == 0 else nc.scalar
            qw = nc.scalar if c % 2 == 0 else nc.sync
            qx.dma_start(out=xT_sb[:, c, :], in_=xT_d[c * 128:(c + 1) * 128, :])
            qw.dma_start(out=wqkvg_sb[:, c, :],
                         in_=wqkvg_d[c * 128:(c + 1) * 128, :])

        for c in range(3):
            load_chunk(c)
        nc.scalar.dma_start(out=cos_sb.rearrange("p a b -> p (a b)"), in_=cos_d)
        nc.sync.dma_start(out=sin_sb.rearrange("p a b -> p (a b)"), in_=sin_d)
        nc.sync.dma_start(out=qgain_sb, in_=qgain_d)
        nc.sync.dma_start(out=gateb_sb, in_=gateb_d)
        nc.scalar.dma_start(out=mask_sb, in_=mask_d)
        for c in range(3, NCHUNK):
            load_chunk(c)
        wproj_sb = consts.tile([128, NHEAD, D], BF16)

        qT_sb = consts.tile([128, NHEAD, T], BF16)   # head-dim-major q
        kT_sb = consts.tile([128, T], BF16)          # head-dim-major k
        v_sb = consts.tile([128, NT, HD + 1], BF16)  # [t | ones] per ki tile
        nc.vector.memset(v_sb[:, :, HD:HD + 1], 1.0)
        yT_sb = consts.tile([128, NHEAD, T], BF16)   # head-dim-major gated y
        glog_all = consts.tile([128, NT, NHEAD], F32)
        egp1_all = consts.tile([128, NT, NHEAD], F32)  # 1 + exp(-glog)
        eps_sb = consts.tile([128, 1], F32)
        nc.vector.memset(eps_sb, EPS)

        # =========== Phase A: QKV + gate, rms stats, rope, transpose ==========
        a_sb = ctx.enter_context(tc.tile_pool(name="phA", bufs=2))

        def emit_tile_post(tt, ti, qkv_a, qkv_b, msq_g, qst_g, kst_g):
            """Vector-side post-processing for one 128-token tile: v copy,
            gate logits, rope on q and k, per-head mean-square stats."""
            nc.vector.tensor_copy(out=v_sb[:, tt, 0:HD], in_=qkv_b[:, 128:256])
            nc.vector.tensor_add(glog_all[:, tt, :], qkv_b[:, 256:260], gateb_sb)

            # rope on q (all 4 heads at once via broadcast cos/sin)
            qa3 = qkv_a.rearrange("p (h d) -> p h d", h=NHEAD)
            x1 = qa3[:, :, 0:HALF]
            x2 = qa3[:, :, HALF:HD]
            cos_t = cos_sb[:, tt, :]
            sin_t = sin_sb[:, tt, :]
            cos_b = bass.AP(tensor=cos_t.tensor, offset=cos_t.offset,
                            ap=[cos_t.ap[0], [0, NHEAD], cos_t.ap[1]])
            sin_b = bass.AP(tensor=sin_t.tensor, offset=sin_t.offset,
                            ap=[sin_t.ap[0], [0, NHEAD], sin_t.ap[1]])
            qrot = qst_g[:, ti, :, :]
            u1 = a_sb.tile([128, NHEAD, HALF], F32, tag="u1")
            u2 = a_sb.tile([128, NHEAD, HALF], F32, tag="u2")
            nc.vector.tensor_mul(u1, x1, cos_b)
            nc.vector.tensor_mul(u2, x2, sin_b)
            nc.vector.tensor_add(qrot[:, :, 0:HALF], u1, u2)
            u3 = a_sb.tile([128, NHEAD, HALF], F32, tag="u3")
            u4 = a_sb.tile([128, NHEAD, HALF], F32, tag="u4")
            nc.vector.tensor_mul(u3, x2, cos_b)
            nc.vector.tensor_mul(u4, x1, sin_b)
            nc.vector.tensor_sub(qrot[:, :, HALF:HD], u3, u4)
            # rope on k
            k1 = qkv_b[:, 0:HALF]
            k2 = qkv_b[:, HALF:HD]
            krot = kst_g[:, ti, :]
            w1 = a_sb.tile([128, HALF], F32, tag="w1")
            w2 = a_sb.tile([128, HALF], F32, tag="w2")
            nc.vector.tensor_mul(w1, k1, cos_t)
            nc.vector.tensor_mul(w2, k2, sin_t)
            nc.vector.tensor_add(krot[:, 0:HALF], w1, w2)
            nc.vector.tensor_mul(w1, k2, cos_t)
            nc.vector.tensor_mul(w2, k1, sin_t)
            nc.vector.tensor_sub(krot[:, HALF:HD], w1, w2)

            # mean-square per head from the (norm-preserving) rotated values
            sqscr = a_sb.tile([128, NHEAD, HD], F32, tag="sqscr")
            sqscr_k = a_sb.tile([128, HD], F32, tag="sqscr_k")
            nc.vector.tensor_mul(sqscr, qrot, qrot)
            nc.vector.tensor_reduce(msq_g[:, ti, 0:NHEAD], sqscr,
                                    axis=mybir.AxisListType.X,
                                    op=mybir.AluOpType.add)
            nc.vector.tensor_mul(sqscr_k, krot, krot)
            nc.vector.tensor_reduce(msq_g[:, ti, NHEAD:NHEAD + 1], sqscr_k,
                                    axis=mybir.AxisListType.X,
                                    op=mybir.AluOpType.add)

        def emit_pair_finish(pr, msq_g, qst_g, kst_g):
            """Rms scale + gain for one pair of tiles + DMA transposes.
            Scalar only runs Sqrt in phase A (table stays resident)."""
            rtmp_g = a_sb.tile([128, 2, NHEAD + 1], F32, tag="rtmp_g")
            nc.scalar.activation(out=rtmp_g, in_=msq_g, func=AF.Sqrt,
                                 scale=1.0 / HD, bias=eps_sb)
            rinv_g = a_sb.tile([128, 2, NHEAD + 1], F32, tag="rinv_g")
            nc.vector.reciprocal(rinv_g, rtmp_g)
            rq_g = a_sb.tile([128, 2, NHEAD], F32, tag="rq_g")
            for ti in range(2):
                nc.vector.tensor_mul(rq_g[:, ti, :], rinv_g[:, ti, 0:NHEAD],
                                     qgain_sb)
            for ti in range(2):
                tt = pr * 2 + ti
                ts = slice(tt * 128, (tt + 1) * 128)
                k_stage = a_sb.tile([128, HD], BF16, tag="k_stage")
                nc.vector.tensor_scalar_mul(k_stage, kst_g[:, ti, :],
                                            rinv_g[:, ti, NHEAD:NHEAD + 1])
                q_stage = a_sb.tile([128, NHEAD, HD], BF16, tag="q_stage")
                for h in range(NHEAD):
                    nc.vector.tensor_scalar_mul(q_stage[:, h, :],
                                                qst_g[:, ti, h, :],
                                                rq_g[:, ti, h:h + 1])
                nc.sync.dma_start_transpose(out=qT_sb[:, :, ts], in_=q_stage)
                nc.sync.dma_start_transpose(out=kT_sb[:, ts], in_=k_stage)

        NPAIR = NT // 2
        with tc.tile_pool(name="phA_ps", bufs=2, space="PSUM") as a_ps:
            # wproj: needed only by phase C; queue behind phase-A loads
            for h in range(NHEAD):
                nc.sync.dma_start(out=wproj_sb[:, h, :],
                                  in_=wproj_d[h * 128:(h + 1) * 128, :])

            # pairs 0..7: chunk-outer, double-buffered; pair 0 tracks the
            # arrival of the input chunk DMAs
            for pr in range(NPAIR):
                qa = a_ps.tile([128, 2, 512], F32, tag="qkv_a")
                qb = a_ps.tile([128, 2, 512], F32, tag="qkv_b")
                for c in range(NCHUNK):
                    for ti in range(2):
                        tt = pr * 2 + ti
                        lhs = xT_sb[:, c, tt * 128:(tt + 1) * 128]
                        nc.tensor.matmul(qa[:, ti, :], lhsT=lhs,
                                         rhs=wqkvg_sb[:, c, 0:512],
                                         start=(c == 0), stop=(c == NCHUNK - 1))
                        nc.tensor.matmul(qb[:, ti, 0:NQKV - 512], lhsT=lhs,
                                         rhs=wqkvg_sb[:, c, 512:NQKV],
                                         start=(c == 0), stop=(c == NCHUNK - 1))
                msq_g = a_sb.tile([128, 2, NHEAD + 1], F32, tag="msq_g")
                qst_g = a_sb.tile([128, 2, NHEAD, HD], BF16, tag="qst_g")
                kst_g = a_sb.tile([128, 2, HD], BF16, tag="kst_g")
                for ti in range(2):
                    emit_tile_post(pr * 2 + ti, ti, qa[:, ti, :],
                                   qb[:, ti, 0:NQKV - 512],
                                   msq_g, qst_g, kst_g)
                emit_pair_finish(pr, msq_g, qst_g, kst_g)

        # =========== Phase B + C: attention, projection =======================
        b_sb = ctx.enter_context(tc.tile_pool(name="phB", bufs=3))
        c_sb = ctx.enter_context(tc.tile_pool(name="phC", bufs=3))

        def emit_C_block(qc, qs, b_ps):
            tt = qc * 4 + qs
            ts = slice(tt * 128, (tt + 1) * 128)
            for nch in range(4):
                o_ps = b_ps.tile([128, 512], F32, tag="o")
                for h in range(NHEAD):
                    nc.tensor.matmul(o_ps, lhsT=yT_sb[:, h, ts],
                                     rhs=wproj_sb[:, h, nch * 512:(nch + 1) * 512],
                                     start=(h == 0), stop=(h == NHEAD - 1))
                o_st = c_sb.tile([128, 512], BF16, tag="o_st")
                if nch % 2 == 0:
                    nc.scalar.copy(out=o_st, in_=o_ps)
                else:
                    nc.vector.tensor_copy(out=o_st, in_=o_ps)
                nc.sync.dma_start(out=out_d[ts, nch * 512:(nch + 1) * 512],
                                  in_=o_st)

        with tc.tile_pool(name="phBC_ps", bufs=2, space="PSUM") as b_ps:
            for qc in range(4):
                nki = 4 * qc + 4

                # gate for this qc's tiles: 1 + exp(-glog) (Exp table resident)
                eg = egp1_all[:, qc * 4:(qc + 1) * 4, :]
                eg_f = eg.rearrange("p a b -> p (a b)")
                nc.scalar.activation(
                    out=eg_f,
                    in_=glog_all[:, qc * 4:(qc + 1) * 4, :].rearrange(
                        "p a b -> p (a b)"),
                    func=AF.Exp, scale=-1.0)
                nc.vector.tensor_scalar_add(eg_f, eg_f, 1.0)

                def emit_S(h, ki):
                    """S^T matmul + exp (+ diagonal mask); returns p tile."""
                    m = ki - 4 * qc
                    nq = 512 - 128 * max(m, 0)
                    q_lo = qc * 512 + 128 * max(m, 0)
                    s_ps = b_ps.tile([128, 512], F32, tag="s")
                    nc.tensor.matmul(s_ps[:, 0:nq],
                                     lhsT=kT_sb[:, ki * 128:(ki + 1) * 128],
                                     rhs=qT_sb[:, h, q_lo:(qc + 1) * 512],
                                     start=True, stop=True)
                    p_sb = b_sb.tile([128, 512], BF16, tag="p")
                    nc.scalar.activation(out=p_sb[:, 0:nq], in_=s_ps[:, 0:nq],
                                         func=AF.Exp, scale=SM_SCALE)
                    if m >= 0:
                        nc.vector.tensor_mul(p_sb[:, 0:128], p_sb[:, 0:128],
                                             mask_sb)
                    return p_sb

                for h in range(NHEAD):
                    y01 = b_ps.tile([128, 2, HD + 1], F32, tag="y01")
                    y23 = b_ps.tile([128, 2, HD + 1], F32, tag="y23")
                    # S emitted one tile ahead of P@V so the tensor queue
                    # always has independent work while exp(ki) runs.
                    p_cur = emit_S(h, 0)
                    for ki in range(nki):
                        p_next = emit_S(h, ki + 1) if ki + 1 < nki else None
                        m = ki - 4 * qc
                        for qs in range(max(m, 0), 4):
                            ytile = y01 if qs < 2 else y23
                            pcol = (qs - max(m, 0)) * 128
                            nc.tensor.matmul(
                                ytile[:, qs % 2, :],
                                lhsT=p_cur[:, pcol:pcol + 128],
                                rhs=v_sb[:, ki, :],
                                start=(ki == 0 and qs % 2 == 0),
                                stop=(ki == 4 * qc + qs and qs % 2 == 1))
                        p_cur = p_next
                    # normalize + fused sigmoid gate + transpose (scalar queue)
                    y_stage = b_sb.tile([128, 4, HD], BF16, tag="y_stage")
                    for qs in range(4):
                        ytile = y01 if qs < 2 else y23
                        tt = qc * 4 + qs
                        den = b_sb.tile([128, 1], F32, tag="den")
                        nc.vector.tensor_mul(den, egp1_all[:, tt, h:h + 1],
                                             ytile[:, qs % 2, HD:HD + 1])
                        sc = b_sb.tile([128, 1], F32, tag="sc")
                        nc.vector.reciprocal(sc, den)
                        nc.vector.tensor_scalar_mul(y_stage[:, qs, :],
                                                    ytile[:, qs % 2, 0:HD], sc)
                    yreg = yT_sb[:, h, qc * 512:(qc + 1) * 512]
                    y3d = bass.AP(tensor=yreg.tensor, offset=yreg.offset,
                                  ap=[yreg.ap[0], [128, 4], [1, 128]])
                    # alternate queues so the last transpose of a qc isn't
                    # stuck behind phase-C output stores
                    yq = nc.sync if h % 2 == 0 else nc.scalar
                    yq.dma_start_transpose(out=y3d, in_=y_stage)

                    # C for the previous qc rides between B head blocks
                    if qc >= 1:
                        emit_C_block(qc - 1, h, b_ps)

            for qs in range(4):
                emit_C_block(3, qs, b_ps)

    nc.compile()
    return nc


def _get_program():
    if "nc" not in _CACHE:
        _CACHE["nc"] = _build_program()
    return _CACHE["nc"]


def _host_prep(x, Wq, Wk, Wv, Wproj, q_gain, gate_w, gate_b):
    """Build the 8 per-core input maps."""
    f = np.float32
    x = np.asarray(x, f)
    WqT = np.asarray(Wq, f).T.astype(NPBF)       # [D, 2048]
    WkT = np.asarray(Wk, f).T.astype(NPBF)       # [D, 512]
    WvT = np.asarray(Wv, f).T.astype(NPBF)
    WpT = np.ascontiguousarray(np.asarray(Wproj, f).T.astype(NPBF))  # [D, D]
    gwT = np.asarray(gate_w, f).T.astype(NPBF)   # [D, 16]
    q_gain = np.asarray(q_gain, f)
    gate_b = np.asarray(gate_b, f)

    inv_freq = 1.0 / (ROPE_BASE ** (np.arange(0, HD, 2, dtype=f) / HD))
    tpos = np.arange(T, dtype=f)
    freqs = np.outer(tpos, inv_freq)
    # device layout [128 partitions, NT tiles, HALF] contiguous
    cos = np.ascontiguousarray(
        np.cos(freqs).astype(f).reshape(NT, 128, HALF).transpose(1, 0, 2)
    ).reshape(128, NT * HALF)
    sin = np.ascontiguousarray(
        np.sin(freqs).astype(f).reshape(NT, 128, HALF).transpose(1, 0, 2)
    ).reshape(128, NT * HALF)

    kloc = np.arange(128)[:, None]
    qloc = np.arange(128)[None, :]
    mask = (qloc >= kloc).astype(NPBF)           # [128, 128]

    xT = [np.ascontiguousarray(x[b].T).astype(NPBF) for b in range(B)]

    in_maps = []
    for core in range(8):
        b, g = divmod(core, 4)
        wqkvg = np.concatenate([
            WqT[:, 512 * g:512 * (g + 1)],
            WkT[:, 128 * g:128 * (g + 1)],
            WvT[:, 128 * g:128 * (g + 1)],
            gwT[:, NHEAD * g:NHEAD * (g + 1)],
        ], axis=1)                               # [D, 772]
        in_maps.append({
            "xT": xT[b],
            "wqkvg": np.ascontiguousarray(wqkvg),
            "wproj": np.ascontiguousarray(WpT[512 * g:512 * (g + 1), :]),
            "cosd": cos,
            "sind": sin,
            "qgain": np.ascontiguousarray(np.broadcast_to(
                q_gain[NHEAD * g:NHEAD * (g + 1)][None, :], (128, NHEAD))),
            "gateb": np.ascontiguousarray(np.broadcast_to(
                gate_b[NHEAD * g:NHEAD * (g + 1)][None, :], (128, NHEAD))),
            "mask": mask,
        })
    return in_maps


def kernel(**inputs):
    nc = _get_program()
    in_maps = _host_prep(**inputs)
    res = run_bass_kernel_spmd(nc, in_maps, list(range(8)))
    parts = [r["out"] for r in res.results]
    out = np.empty((B, T, D), np.float32)
    for b in range(B):
        out[b] = (parts[4 * b].astype(np.float32)
                  + parts[4 * b + 1].astype(np.float32)
                  + parts[4 * b + 2].astype(np.float32)
                  + parts[4 * b + 3].astype(np.float32))
    return out
